# revision 1
# baseline (speedup 1.0000x reference)
"""AdaptivePiecewiseConv2d forward on 8 Trainium2 NeuronCores.

Math: the reference computes, for each im2col row n and output channel o,
    out[n,o] = sum_i f_{i,o}(X[n,i])
where f_{i,o} is a P=3-knot piecewise-linear function with knots
pos[i,o,:], values val[i,o,:].  f is continuous at the middle knot p1, so
    f(x) = a1*x + b1 + relu(x - p1) * (a2 - a1)
with a1, a2 the segment slopes and b1 the first-segment intercept.  When
the middle knot p1 is shared across edges (true for the 'uniform' position
init: knots are the same linspace everywhere), the layer factorizes into
two matmuls plus a bias:
    out = X @ A1 + relu(X - p1) @ (A2 - A1) + bias,  bias[o] = sum_i b1[i,o]

Sharding: pure data parallel over im2col rows N = B*H*W = 8192.  Core k
handles (b = k//4, y in [16*(k%4), 16*(k%4)+16)).

Key layout trick: each core's padded x slab is shipped as x3b[(c,kw), y*64+x]
(the kw in {0,1,2} horizontal shifts stacked per channel - a pure reshape of
the halo-replicated slab, pre-cast to bf16).  In that layout every kh window
of the im2col is a CONTIGUOUS free-dim slice x3b[:, kh*64 : kh*64+1024], so
the matmuls read x (and relu(x-p1)) directly - no on-chip im2col at all.
The knot tables are likewise packed onto partitions 0:48 (columns =
[pos|val][knot][kh][o]) so a single DVE op chain computes every kh block's
slopes at partition base 0 - no partition-shift fixups anywhere.  The
contraction runs as 3 chunks of K=48 (one per kh) per term, in bf16, with
2-way PE column tiling (both 512-column output halves concurrently).  The
engine programs are written in raw bacc (manual semaphores, no Tile
scheduler) to keep the dependency chain and kernel tail minimal.
"""

import ml_dtypes
import numpy as np

B, CIN, H, W = 2, 16, 64, 64
COUT, P = 64, 3
K = 3
I_TOT = CIN * K * K  # 144
N_CORES = 8
ROWS_PER_CORE = 16  # y-rows of the output image per core
N_LOC = ROWS_PER_CORE * W  # 1024 im2col rows per core
KCH = K * CIN  # 48 contraction rows per kh chunk
XFREE = (ROWS_PER_CORE + 2) * W  # 1152
KO = K * COUT  # 192 columns per knot block

_STATE = {}


def _install_prof_shim():
    """Make run_bass_kernel_spmd(trace=True) safe in images missing
    antenv.axon_hooks; harmless no-op if anything is absent."""
    try:
        import sys, types

        if "antenv.axon_hooks" not in sys.modules:
            mod = types.ModuleType("antenv.axon_hooks")
            holder = [None]
            mod.set_axon_ntff_profile_hook = lambda h: holder.__setitem__(0, h)
            mod.get_axon_ntff_profile_hook = lambda: holder[0]
            sys.modules["antenv.axon_hooks"] = mod
            import antenv

            antenv.axon_hooks = mod
            try:
                from trn_agent_boot.trn_boot import _ntff_profile_via_ctypes

                hook = _ntff_profile_via_ctypes("/opt/axon/libaxon_pjrt.so")
                mod.set_axon_ntff_profile_hook(hook)
            except Exception:
                pass
        import concourse.bass_utils as bu

        if getattr(bu.upload_artifacts, "__name__", "") != "<lambda>":
            bu.upload_artifacts = lambda tmpdir: tmpdir
    except Exception:
        pass


def _build_program(width):
    import concourse.bass as bass
    import concourse.mybir as mybir
    from concourse import bacc

    f32 = mybir.dt.float32
    bf16 = mybir.dt.bfloat16
    sub = mybir.AluOpType.subtract
    mult = mybir.AluOpType.mult
    add_op = mybir.AluOpType.add
    act_id = mybir.ActivationFunctionType.Identity
    act_relu = mybir.ActivationFunctionType.Relu
    inv_w = float(1.0 / width)

    nc = bacc.Bacc(
        "TRN2", target_bir_lowering=False, num_devices=N_CORES,
        enable_partition_id=False,
    )
    x_d = nc.dram_tensor("x3b", [KCH, XFREE], bf16, kind="ExternalInput")
    tbl_d = nc.dram_tensor("tbl3", [KCH, 2 * K * KO], f32, kind="ExternalInput")
    out_d = nc.dram_tensor("out", [128, 512], bf16, kind="ExternalOutput")

    from contextlib import ExitStack

    with ExitStack() as ctx:
        e = ctx.enter_context
        xbf = e(nc.sbuf_tensor([KCH, XFREE], bf16))
        tblv = e(nc.sbuf_tensor([KCH, K * KO], f32))
        tblp = e(nc.sbuf_tensor([KCH, K * KO], f32))
        negp1 = e(nc.sbuf_tensor([KCH, 1], f32))
        pos3 = e(nc.sbuf_tensor([KCH, XFREE], bf16))
        a0f = e(nc.sbuf_tensor([KCH, KO], f32))
        a2bf = e(nc.sbuf_tensor([KCH, KO], bf16))
        w1_all = e(nc.sbuf_tensor([KCH, KO], bf16))
        wd_all = e(nc.sbuf_tensor([KCH, KO], bf16))
        tmp = e(nc.sbuf_tensor([KCH, KO], f32))
        b1 = e(nc.sbuf_tensor([KCH, KO], bf16))
        ones = e(nc.sbuf_tensor([KCH, 1], bf16))
        bias = e(nc.sbuf_tensor([128, 1], f32))
        ob = e(nc.sbuf_tensor([128, 512], bf16))
        psA = e(nc.psum_tensor([128, 512], f32))
        psb = e(nc.psum_tensor([128, 1], f32))
        s_in = e(nc.semaphore("s_in"))
        s_x = e(nc.semaphore("s_x"))
        s_v2 = e(nc.semaphore("s_v2"))
        s_tp = e(nc.semaphore("s_tp"))
        s_np = e(nc.semaphore("s_np"))
        s_w = e(nc.semaphore("s_w"))
        s_p3 = e(nc.semaphore("s_p3"))
        s_b1 = e(nc.semaphore("s_b1"))
        s_mm = e(nc.semaphore("s_mm"))
        s_mmn = e(nc.semaphore("s_mmn"))
        s_bc = e(nc.semaphore("s_bc"))
        s_ev = e(nc.semaphore("s_ev"))
        s_out = e(nc.semaphore("s_out"))
        block = e(nc.Block())

        posv = tblp.ap().rearrange("p (k x) -> p k x", k=K)
        valv = tblv.ap().rearrange("p (k x) -> p k x", k=K)

        @block.sync
        def _(sync):
            # val knots 0+1 first (they make w1, which gates matmul 1), then
            # x, then val knot 2 (only needed for the wd weights)
            sync.dma_start(
                out=tblv[:, 0 : 2 * KO], in_=tbl_d.ap()[:, K * KO : K * KO + 2 * KO]
            ).then_inc(s_in, 16)
            sync.dma_start(out=xbf[:], in_=x_d.ap()[:]).then_inc(s_x, 16)
            sync.dma_start(
                out=tblv[:, 2 * KO :], in_=tbl_d.ap()[:, K * KO + 2 * KO :]
            ).then_inc(s_v2, 16)
            sync.wait_ge(s_ev, 1)
            sync.dma_start(out=out_d.ap()[:], in_=ob[:]).then_inc(s_out, 16)
            sync.wait_ge(s_out, 16)

        @block.scalar
        def _(scalar):
            scalar.dma_start(out=tblp[:], in_=tbl_d.ap()[:, 0 : K * KO]).then_inc(
                s_tp, 16
            )
            # relu(x - p1): needs x (second sync DMA) + negp1 (DVE)
            scalar.wait_ge(s_x, 16)
            scalar.wait_ge(s_np, 1)
            nc.scalar.activation(pos3[:], xbf[:], act_relu, bias=negp1[:]).then_inc(
                s_p3, 1
            )
            # eviction: needs the main matmuls + the copied bias (single ACT
            # op: concurrent ACT+DVE reads of one PSUM bank crash the device)
            scalar.wait_ge(s_mmn, 1)
            scalar.wait_ge(s_bc, 1)
            nc.scalar.activation(
                ob[:], psA.ap()[:], act_id, bias=bias[:], scale=inv_w
            ).then_inc(s_ev, 1)

        @block.vector
        def _(vector):
            vector.wait_ge(s_tp, 16)
            nc.vector.tensor_scalar_mul(negp1[:], tblp[:, KO : KO + 1], -1.0).then_inc(
                s_np, 1
            )
            vector.wait_ge(s_in, 16)  # val knots 0+1 landed
            # slopes cast to bf16 directly in the subtract (w1 gates MM 1)
            nc.vector.tensor_tensor(
                w1_all[:], valv[:, 1, :], valv[:, 0, :], sub
            ).then_inc(s_w, 1)
            vector.wait_ge(s_v2, 16)  # val knot 2 landed
            nc.vector.tensor_tensor(a2bf[:], valv[:, 2, :], valv[:, 1, :], sub)
            nc.vector.tensor_tensor(wd_all[:], a2bf[:], w1_all[:], sub).then_inc(
                s_w, 1
            )
            # bias path in f32 (off the critical chain)
            nc.vector.tensor_tensor(a0f[:], valv[:, 1, :], valv[:, 0, :], sub)
            nc.vector.tensor_tensor(tmp[:], posv[:, 1, :], a0f[:], mult)
            nc.vector.tensor_scalar_mul(tmp[:], tmp[:], inv_w)
            nc.vector.tensor_tensor(b1[:], valv[:, 1, :], tmp[:], sub)
            nc.vector.memset(ones[:], 1.0).then_inc(s_b1, 1)
            vector.wait_ge(s_mm, 1)  # bias matmuls complete
            nc.vector.tensor_copy(bias[:], psb.ap()[:]).then_inc(s_bc, 1)

        @block.tensor
        def _(tensor):
            w1g = [w1_all.ap()[:, g * COUT : (g + 1) * COUT] for g in range(K)]
            wdg = [wd_all.ap()[:, g * COUT : (g + 1) * COUT] for g in range(K)]

            def mm_pair(wt, rhs_t, kh, start, stop):
                last = None
                for cg in (0, COUT):
                    base = kh * W + (cg // COUT) * 512
                    last = nc.tensor.matmul(
                        psA.ap()[cg : cg + COUT, :],
                        wt,
                        rhs_t[:, base : base + 512],
                        start=start,
                        stop=stop,
                        tile_position=(0, cg),
                        skip_group_check=True,
                    )
                return last

            tensor.wait_ge(s_x, 16)  # x landed
            tensor.wait_ge(s_w, 1)  # w1_all ready
            mm_pair(w1g[0], xbf.ap(), 0, True, False)
            mm_pair(w1g[1], xbf.ap(), 1, False, False)
            mm_pair(w1g[2], xbf.ap(), 2, False, False)
            tensor.wait_ge(s_w, 2)  # wd_all ready
            tensor.wait_ge(s_p3, 1)  # pos3 ready
            mm_pair(wdg[0], pos3.ap(), 0, False, False)
            mm_pair(wdg[1], pos3.ap(), 1, False, False)
            # bias matmuls squeezed in before the last pair: the bias copy
            # then overlaps it instead of trailing it
            tensor.wait_ge(s_b1, 1)
            for cg in (0, COUT):
                for g in range(K):
                    ins = nc.tensor.matmul(
                        psb.ap()[cg : cg + COUT, :],
                        b1[:, g * COUT : (g + 1) * COUT],
                        ones[:],
                        start=(g == 0),
                        stop=(g == K - 1),
                        tile_position=(0, cg),
                        skip_group_check=True,
                    )
            ins.then_inc(s_mm, 1)
            mm_pair(wdg[2], pos3.ap(), 2, False, True).then_inc(s_mmn, 1)

    nc.compile()
    return nc


def _fast_path_ok(positions):
    if positions.shape != (I_TOT, COUT, P):
        return False
    p = positions
    # middle knot must be shared across all edges; knots strictly sorted with
    # one uniform segment width (true for any linspace position init)
    if np.ptp(p[:, :, 1]) != 0.0:
        return False
    w01 = p[:, :, 1] - p[:, :, 0]
    w12 = p[:, :, 2] - p[:, :, 1]
    w = w01.flat[0]
    if w <= 0.0 or np.ptp(w01) != 0.0 or np.any(w12 != w):
        return False
    return True


def _reference_numpy(x, positions, values):
    xf = x.astype(np.float32)
    Bs, C, Hs, Ws = xf.shape
    xp = np.pad(xf, ((0, 0), (0, 0), (1, 1), (1, 1)))
    cols = [xp[:, :, i : i + Hs, j : j + Ws] for i in range(K) for j in range(K)]
    pch = np.stack(cols, 2).reshape(Bs, C * K * K, Hs * Ws)
    X = pch.transpose(0, 2, 1).reshape(-1, C * K * K)
    Np, Ii = X.shape
    Pp = positions.shape[-1]
    out = np.zeros((Np, positions.shape[1]), np.float32)
    chunk = 1024
    for st in range(0, Np, chunk):
        xb = X[st : st + chunk, :, None]
        idx = np.sum(xb[..., None] >= positions[None], axis=-1)
        idx = np.clip(idx, 1, Pp - 1)
        f = np.zeros((xb.shape[0], Ii, positions.shape[1]), np.float32)
        for s in range(1, Pp):
            x0 = positions[:, :, s - 1]
            x1 = positions[:, :, s]
            y0 = values[:, :, s - 1]
            y1 = values[:, :, s]
            t = (xb - x0) / (x1 - x0)
            f = np.where(idx == s, y0 + t * (y1 - y0), f)
        out[st : st + chunk] = f.sum(axis=1)
    O = out.shape[-1]
    return out.reshape(Bs, Hs * Ws, O).transpose(0, 2, 1).reshape(Bs, O, Hs, Ws)


def kernel(x, positions, values):
    x = np.ascontiguousarray(x, dtype=np.float32)
    positions = np.ascontiguousarray(positions, dtype=np.float32)
    values = np.ascontiguousarray(values, dtype=np.float32)

    if not _fast_path_ok(positions):
        # pathological tables (unsorted / non-uniform knots): bit-exact
        # reference emulation on host
        return _reference_numpy(x, positions, values)

    _install_prof_shim()
    from concourse.bass_utils import run_bass_kernel_spmd

    width = float(positions[0, 0, 1] - positions[0, 0, 0])
    key = ("nc", width)
    if key not in _STATE:
        _STATE[key] = _build_program(width)
    nc = _STATE[key]

    # tbl3[c*3+kw, t*576 + k*192 + kh*64 + o]
    pos5 = positions.reshape(CIN, K, K, COUT, P).transpose(0, 2, 4, 1, 3)
    val5 = values.reshape(CIN, K, K, COUT, P).transpose(0, 2, 4, 1, 3)
    tbl = np.ascontiguousarray(
        np.concatenate(
            [pos5.reshape(KCH, K * K * COUT), val5.reshape(KCH, K * K * COUT)], axis=1
        )
    )

    xp = np.pad(x, ((0, 0), (0, 0), (1, 1), (1, 1)))
    in_maps = []
    for k in range(N_CORES):
        b, y0 = divmod(k, N_CORES // B)
        y0 *= ROWS_PER_CORE
        slab = xp[b, :, y0 : y0 + ROWS_PER_CORE + 2, :]  # [16, 18, 66]
        x3 = np.empty((CIN, K, ROWS_PER_CORE + 2, W), np.float32)
        for kw in range(K):
            x3[:, kw] = slab[:, :, kw : kw + W]
        in_maps.append(
            {"x3b": x3.reshape(KCH, XFREE).astype(ml_dtypes.bfloat16), "tbl3": tbl}
        )

    res = run_bass_kernel_spmd(nc, in_maps, core_ids=list(range(N_CORES)))
    _STATE["last_result"] = res

    out = np.empty((B, COUT, H, W), np.float32)
    for k in range(N_CORES):
        b, y0 = divmod(k, N_CORES // B)
        y0 *= ROWS_PER_CORE
        o2 = (
            res.results[k]["out"].astype(np.float32).reshape(2, COUT, 512)
            .transpose(1, 0, 2)
        )
        out[b, :, y0 : y0 + ROWS_PER_CORE, :] = o2.reshape(COUT, ROWS_PER_CORE, W)
    return out



# revision 3
# speedup vs baseline: 1.0547x; 1.0547x over previous
"""AdaptivePiecewiseConv2d forward on 8 Trainium2 NeuronCores.

Math: for each im2col row n and output channel o,
    out[n,o] = sum_i f_{i,o}(X[n,i])
with f a P=3-knot piecewise-linear function (knots pos[i,o,:], values
val[i,o,:]).  f is continuous at the middle knot p1, so
    f(x) = a1*x + b1 + relu(x - p1) * (a2 - a1)
a1 = (v1-v0)/(p1-p0), a2 = (v2-v1)/(p2-p1), b1 = v1 - p1*a1.  When p1 is
shared across every (i,o) (true for any linspace position init) the layer
factorizes into two matmuls plus a bias:
    out = X @ A1 + relu(X - p1) @ (A2 - A1) + bias,   bias[o] = sum_i b1[i,o]

All table math (slopes, bias) is host-precomputed — it is weight
preparation, O(I*O) tiny — so the device kernel only sees two bf16 weight
blocks.  The bias is folded into the first matmul with a ones-row appended
to x (row 48), so the device graph is exactly:

    DMA x, DMA w  →  relu (DVE)  →  12 matmuls (PE)  →  evict (ACT)  →  DMA out

Sharding: pure data parallel over im2col rows N = B*H*W = 8192.  Core k
handles (b = k//4, y in [16*(k%4), 16*(k%4)+16)).

Layout trick (same as the earlier kernel): the padded x slab is shipped as
x49[(c,kw), y*64+x] with the 3 kw-shifts stacked per channel, so every kh
window of the im2col is a contiguous free-dim slice x49[:, kh*64:kh*64+1024]
— no on-chip im2col.  Contraction runs as 3 chunks of K=49 (term 1, with
the ones/bias row) resp. K=48 (term 2) per kh, in bf16, with 2-way PE
column tiling (both 512-column output halves concurrently).

Perf notes (from the ntff trace + the CoreSim cost model):
 - The PE runs at 1.2 GHz until it has been busy ~3 us, then 2.4 GHz.  A
   chain of dummy warm-up matmuls (into a scratch PSUM bank) runs while
   the input DMAs are in flight so the real matmuls hit full clock.
 - DMA completion → engine semaphore visibility is ~0.9 us, and each
   DMA_DIRECT2D issue costs ~0.65 us of sequencer time, so the two input
   DMAs are issued in parallel from SP and ACT at program start.
 - The ucode dispatcher's epilogue (it resets all 253 device semaphores,
   ~7 us) runs inside the measured window and is not controllable from
   the kernel; everything else is minimized around it.
"""

import ml_dtypes
import numpy as np

B, CIN, H, W = 2, 16, 64, 64
COUT, P = 64, 3
K = 3
I_TOT = CIN * K * K  # 144
N_CORES = 8
ROWS_PER_CORE = 16  # y-rows of the output image per core
N_LOC = ROWS_PER_CORE * W  # 1024 im2col rows per core
KCH = K * CIN  # 48 contraction rows per kh chunk
KCH1 = KCH + 1  # + the ones/bias row
XFREE = (ROWS_PER_CORE + 2) * W  # 1152
WARMUP_MM = 6  # dummy matmuls to ramp the PE p-state while DMAs fly
WAIT_OUT = True  # sync waits for the output DMA before the end barrier

_STATE = {}


def _install_prof_shim():
    """Make run_bass_kernel_spmd(trace=True) safe in images missing
    antenv.axon_hooks; harmless no-op if anything is absent."""
    try:
        import sys, types

        if "antenv.axon_hooks" not in sys.modules:
            mod = types.ModuleType("antenv.axon_hooks")
            holder = [None]
            mod.set_axon_ntff_profile_hook = lambda h: holder.__setitem__(0, h)
            mod.get_axon_ntff_profile_hook = lambda: holder[0]
            sys.modules["antenv.axon_hooks"] = mod
            import antenv

            antenv.axon_hooks = mod
            try:
                from trn_agent_boot.trn_boot import _ntff_profile_via_ctypes

                hook = _ntff_profile_via_ctypes("/opt/axon/libaxon_pjrt.so")
                mod.set_axon_ntff_profile_hook(hook)
            except Exception:
                pass
        import concourse.bass_utils as bu

        if getattr(bu.upload_artifacts, "__name__", "") != "<lambda>":
            bu.upload_artifacts = lambda tmpdir: tmpdir
    except Exception:
        pass


def _build_program(p1):
    import concourse.bass as bass
    import concourse.mybir as mybir
    from concourse import bacc

    f32 = mybir.dt.float32
    bf16 = mybir.dt.bfloat16
    sub = mybir.AluOpType.subtract
    mx = mybir.AluOpType.max
    act_id = mybir.ActivationFunctionType.Identity

    nc = bacc.Bacc(
        "TRN2", target_bir_lowering=False, num_devices=N_CORES,
        enable_partition_id=False,
    )
    x_d = nc.dram_tensor("x49", [KCH1, XFREE], bf16, kind="ExternalInput")
    w_d = nc.dram_tensor("wt", [KCH1, 2 * K * COUT], bf16, kind="ExternalInput")
    out_d = nc.dram_tensor("out", [128, 512], bf16, kind="ExternalOutput")

    from contextlib import ExitStack

    with ExitStack() as ctx:
        e = ctx.enter_context
        xbf = e(nc.sbuf_tensor([KCH1, XFREE], bf16))
        wt = e(nc.sbuf_tensor([KCH1, 2 * K * COUT], bf16))
        pos3 = e(nc.sbuf_tensor([KCH, XFREE], bf16))
        ob = e(nc.sbuf_tensor([128, 512], bf16))
        scratch = e(nc.sbuf_tensor([64, 576], bf16))
        psA = e(nc.psum_tensor([128, 512], f32))
        psW = e(nc.psum_tensor([64, 512], f32))
        s_x = e(nc.semaphore("s_x"))
        s_w = e(nc.semaphore("s_w"))
        s_p3 = e(nc.semaphore("s_p3"))
        s_mm = e(nc.semaphore("s_mm"))
        s_ev = e(nc.semaphore("s_ev"))
        s_out = e(nc.semaphore("s_out"))
        block = e(nc.Block())

        @block.sync
        def _(sync):
            sync.dma_start(out=xbf[:], in_=x_d.ap()[:]).then_inc(s_x, 16)
            sync.wait_ge(s_ev, 1)
            sync.dma_start(out=out_d.ap()[:], in_=ob[:]).then_inc(s_out, 16)
            if WAIT_OUT:
                sync.wait_ge(s_out, 16)

        @block.scalar
        def _(scalar):
            scalar.dma_start(out=wt[:], in_=w_d.ap()[:]).then_inc(s_w, 16)
            # eviction: PSUM -> SBUF bf16 (single ACT op over all 128
            # partitions; both column groups)
            scalar.wait_ge(s_mm, 1)
            nc.scalar.activation(ob[:], psA.ap()[:], act_id).then_inc(s_ev, 1)

        @block.vector
        def _(vector):
            # relu(x - p1) for the second term (only the 48 real rows)
            vector.wait_ge(s_x, 16)
            nc.vector.tensor_scalar(
                pos3[:], xbf.ap()[0:KCH, :], float(p1), 0.0, sub, mx
            ).then_inc(s_p3, 1)

        @block.tensor
        def _(tensor):
            # p-state warm-up: garbage matmuls into a scratch PSUM bank
            for _i in range(WARMUP_MM):
                nc.tensor.matmul(
                    psW.ap()[:, :],
                    scratch.ap()[:, 0:COUT],
                    scratch.ap()[:, 64:576],
                    start=True,
                    stop=True,
                    tile_position=(0, 0),
                    skip_group_check=True,
                )
            tensor.wait_ge(s_w, 16)
            tensor.wait_ge(s_x, 16)

            def mm(w_ap, rhs_ap, kh, cg, start, stop):
                base = kh * W + (cg // COUT) * 512
                return nc.tensor.matmul(
                    psA.ap()[cg : cg + COUT, :],
                    w_ap,
                    rhs_ap[:, base : base + 512],
                    start=start,
                    stop=stop,
                    tile_position=(0, cg),
                    skip_group_check=True,
                )

            # term 1: x @ A1 (+ bias via the ones row, kh=0 chunk only)
            for kh in range(K):
                for cg in (0, COUT):
                    mm(
                        wt.ap()[0:KCH1, kh * COUT : (kh + 1) * COUT],
                        xbf.ap()[0:KCH1, :],
                        kh,
                        cg,
                        start=(kh == 0),
                        stop=False,
                    )
            # term 2: relu(x - p1) @ (A2 - A1)
            tensor.wait_ge(s_p3, 1)
            for kh in range(K):
                for cg in (0, COUT):
                    ins = mm(
                        wt.ap()[0:KCH, (K + kh) * COUT : (K + kh + 1) * COUT],
                        pos3.ap()[0:KCH, :],
                        kh,
                        cg,
                        start=False,
                        stop=(kh == K - 1),
                    )
            ins.then_inc(s_mm, 1)

    nc.compile()
    return nc


def _fast_path_ok(positions):
    if positions.shape != (I_TOT, COUT, P):
        return False
    p = positions
    # middle knot shared across all edges; knots strictly sorted
    if np.ptp(p[:, :, 1]) != 0.0:
        return False
    if np.any(p[:, :, 1] <= p[:, :, 0]) or np.any(p[:, :, 2] <= p[:, :, 1]):
        return False
    return True


def _reference_numpy(x, positions, values):
    xf = x.astype(np.float32)
    Bs, C, Hs, Ws = xf.shape
    xp = np.pad(xf, ((0, 0), (0, 0), (1, 1), (1, 1)))
    cols = [xp[:, :, i : i + Hs, j : j + Ws] for i in range(K) for j in range(K)]
    pch = np.stack(cols, 2).reshape(Bs, C * K * K, Hs * Ws)
    X = pch.transpose(0, 2, 1).reshape(-1, C * K * K)
    Np, Ii = X.shape
    Pp = positions.shape[-1]
    out = np.zeros((Np, positions.shape[1]), np.float32)
    chunk = 1024
    for st in range(0, Np, chunk):
        xb = X[st : st + chunk, :, None]
        idx = np.sum(xb[..., None] >= positions[None], axis=-1)
        idx = np.clip(idx, 1, Pp - 1)
        f = np.zeros((xb.shape[0], Ii, positions.shape[1]), np.float32)
        for s in range(1, Pp):
            x0 = positions[:, :, s - 1]
            x1 = positions[:, :, s]
            y0 = values[:, :, s - 1]
            y1 = values[:, :, s]
            t = (xb - x0) / (x1 - x0)
            f = np.where(idx == s, y0 + t * (y1 - y0), f)
        out[st : st + chunk] = f.sum(axis=1)
    O = out.shape[-1]
    return out.reshape(Bs, Hs * Ws, O).transpose(0, 2, 1).reshape(Bs, O, Hs, Ws)


def _chunk_layout(a):
    # [144, 64] (i = c*9 + kh*3 + kw) -> [48, 192] (row c*3+kw, col kh*64+o)
    return np.ascontiguousarray(
        a.reshape(CIN, K, K, COUT).transpose(0, 2, 1, 3).reshape(KCH, K * COUT)
    )


def kernel(x, positions, values):
    x = np.ascontiguousarray(x, dtype=np.float32)
    positions = np.ascontiguousarray(positions, dtype=np.float32)
    values = np.ascontiguousarray(values, dtype=np.float32)

    if not _fast_path_ok(positions):
        # pathological tables (unsorted / varying middle knot): bit-exact
        # reference emulation on host
        return _reference_numpy(x, positions, values)

    _install_prof_shim()
    from concourse.bass_utils import run_bass_kernel_spmd

    p1 = float(positions[0, 0, 1])
    key = ("nc", p1)
    if key not in _STATE:
        _STATE[key] = _build_program(p1)
    nc = _STATE[key]

    # host weight prep: per-edge slopes and the folded bias
    p0 = positions[:, :, 0]
    p2 = positions[:, :, 2]
    v0 = values[:, :, 0]
    v1 = values[:, :, 1]
    v2 = values[:, :, 2]
    a1 = (v1 - v0) / (p1 - p0)
    a2 = (v2 - v1) / (p2 - p1)
    bias = (v1 - p1 * a1).sum(axis=0)  # [64]
    wt = np.zeros((KCH1, 2 * K * COUT), np.float32)
    wt[0:KCH, 0 : K * COUT] = _chunk_layout(a1)
    wt[0:KCH, K * COUT :] = _chunk_layout(a2 - a1)
    wt[KCH, 0:COUT] = bias
    wt = wt.astype(ml_dtypes.bfloat16)

    xp = np.pad(x, ((0, 0), (0, 0), (1, 1), (1, 1)))
    in_maps = []
    for k in range(N_CORES):
        b, y0 = divmod(k, N_CORES // B)
        y0 *= ROWS_PER_CORE
        slab = xp[b, :, y0 : y0 + ROWS_PER_CORE + 2, :]  # [16, 18, 66]
        x49 = np.empty((KCH1, XFREE), np.float32)
        x3 = x49[0:KCH].reshape(CIN, K, ROWS_PER_CORE + 2, W)
        for kw in range(K):
            x3[:, kw] = slab[:, :, kw : kw + W]
        x49[KCH] = 1.0
        in_maps.append(
            {"x49": x49.astype(ml_dtypes.bfloat16), "wt": wt}
        )

    res = run_bass_kernel_spmd(nc, in_maps, core_ids=list(range(N_CORES)))
    _STATE["last_result"] = res

    out = np.empty((B, COUT, H, W), np.float32)
    for k in range(N_CORES):
        b, y0 = divmod(k, N_CORES // B)
        y0 *= ROWS_PER_CORE
        o2 = (
            res.results[k]["out"].astype(np.float32).reshape(2, COUT, 512)
            .transpose(1, 0, 2)
        )
        out[b, :, y0 : y0 + ROWS_PER_CORE, :] = o2.reshape(COUT, ROWS_PER_CORE, W)
    return out
